# revision 32
# baseline (speedup 1.0000x reference)
"""LoRA MLP (gate_up + SiLU*up + down, each with rank-16 LoRA) on 8 TRN2 cores.

Strategy: data-parallel over tokens (16384 = 8 x 2048); weights replicated to
every core, no collectives. All tensors bf16 (PE full rate + FWL fast weight
load + half DMA/SBUF), fp32 PSUM accumulation, fp32 output.

The rank-16 LoRA is folded into the base weights on device:
    W1' = W_gate_up + A_gate_up @ B_gate_up   (PE matmul K=16 + DVE add)
    W2' = W_down    + A_down    @ B_down
so the steady-state loop is a pure dense MLP: clean 8-deep / 22-deep PSUM
accumulation chains with no 16-row LoRA matmuls serializing the PE.

Because the PE instruction queue is strict FIFO, fold work is emitted one
group-pair AHEAD of its consumer and chopped into small units (2 matmuls +
1 DVE add) that are interleaved between the main matmul chunks — by the time
the PE reaches a fold matmul its PSUM slot is long free, so the PE never
stalls on the fold's DVE evacuation chain (which caused HAM re-throttling).

Per core: 2 blocks of 1024 tokens. W2' (44KB/partition) stays SBUF-resident;
W1' groups are folded during block 0 and round-tripped through a DRAM scratch
for block 1 (write and read share one FIFO DMA queue, which orders them).
Activations stay in [feature, token] layout; each LDWEIGHTS feeds 2 matmuls.
DMA spread: weights on sync, x on vector, consts+W2raw on gpsimd, out on
scalar. PSUM: gate 2 + up 2 + shared fold/down-proj 4 banks.
"""

from collections import deque

import numpy as np
import ml_dtypes

import concourse.mybir as mybir
import concourse.tile as tile
from concourse import bacc
from concourse.bass_utils import run_bass_kernel_spmd

TOKENS, D, FF, R = 16384, 1024, 2816, 16
N_CORES = 8
T_CORE = TOKENS // N_CORES  # 2048
BLK = 1024                  # tokens per block (2 blocks/core)
TS = 512                    # psum free-dim tile (1 bank fp32)
DT = D // 128               # 8 d-model tiles
FFT = FF // 128             # 22 ff tiles
# W1' fold groups: per half (gate/up) five 512-col groups + one 256-col tail;
# groups never straddle the gate/up boundary so their lifetimes stay local.
NG = 12
GDEF = [(512 * g, 512) for g in range(5)] + [(2560, 256)]
GDEF += [(2816 + s, w) for s, w in GDEF[:6]]
F32 = mybir.dt.float32
BF16 = mybir.dt.bfloat16
SILU = mybir.ActivationFunctionType.Silu
COPY = mybir.ActivationFunctionType.Copy
BF = ml_dtypes.bfloat16

_prog_cache = {}


def _build():
    nc = bacc.Bacc("TRN2", target_bir_lowering=False, debug=False)
    xT = nc.dram_tensor("xT", [D, T_CORE], BF16, kind="ExternalInput").ap()
    # W1G/W2G are host-pre-grouped so every weight DMA is contiguous per partition
    w1g = nc.dram_tensor("W1G", [NG, 128, DT, 512], BF16, kind="ExternalInput").ap()
    a1t = nc.dram_tensor("A1T", [R, D], BF16, kind="ExternalInput").ap()
    b1 = nc.dram_tensor("B_gate_up", [R, 2 * FF], BF16, kind="ExternalInput").ap()
    w2g = nc.dram_tensor("W2G", [FFT, 128, D], BF16, kind="ExternalInput").ap()
    a2t = nc.dram_tensor("A2T", [R, FF], BF16, kind="ExternalInput").ap()
    b2 = nc.dram_tensor("B_down", [R, D], BF16, kind="ExternalInput").ap()
    out = nc.dram_tensor("out", [T_CORE, D], F32, kind="ExternalOutput").ap()
    # W1' spill space so block 1 re-reads the folded weights instead of refolding
    w1s = nc.dram_tensor("w1s", [NG, 128, DT, 512], BF16, kind="Internal").ap()

    xTr = xT.rearrange("(dt p) t -> p dt t", p=128)   # [128, 8, 2048]

    with tile.TileContext(nc) as tc:
        with (
            tc.tile_pool(name="constp", bufs=1) as constp,
            tc.tile_pool(name="w1c", bufs=4) as w1c,      # W1' group tiles, 8KB each
            tc.tile_pool(name="w2p", bufs=1) as w2p,      # W2' resident
            tc.tile_pool(name="w1raw", bufs=2) as w1raw,
            tc.tile_pool(name="w2raw", bufs=2) as w2raw,
            tc.tile_pool(name="b1p", bufs=2) as b1p,
            tc.tile_pool(name="xp", bufs=2) as xp,
            tc.tile_pool(name="hp", bufs=1) as hp,
            tc.tile_pool(name="tmpp", bufs=2) as tmpp,
            tc.tile_pool(name="evp", bufs=2) as evp,
            tc.tile_pool(name="ps", bufs=1, space="PSUM") as ps,
        ):
            a1t_sb = constp.tile([R, D], BF16)
            nc.gpsimd.dma_start(a1t_sb[:], a1t[:])
            b2_sb = constp.tile([R, D], BF16)
            nc.gpsimd.dma_start(b2_sb[:], b2[:])
            w2sb = w2p.tile([128, FFT, D], BF16)

            pending = deque()  # fold micro-units, drained between main MM chunks

            def drain(n):
                for _ in range(min(n, len(pending))):
                    pending.popleft()()

            def sched_fold_w1(g):
                """Queue fold of W1' group g (8 units of one dt each + spill)."""
                c0, w = GDEF[g]
                raw = w1raw.tile([128, DT, 512], BF16, tag="w1raw")
                nc.sync.dma_start(raw[:, :, 0:w], w1g[g][:, :, 0:w])
                b1c = b1p.tile([R, 512], BF16, tag="b1c")
                nc.sync.dma_start(b1c[:, 0:w], b1[:, c0 : c0 + w])
                wt = w1c.tile([128, DT, 512], BF16, tag="w1c")

                def unit(dt):
                    pf = ps.tile([128, TS], F32, tag="pf", bufs=4, name="pf")
                    nc.tensor.matmul(
                        pf[:, 0:w],
                        a1t_sb[:, dt * 128 : (dt + 1) * 128],
                        b1c[:, 0:w],
                        start=True, stop=True,
                    )
                    nc.vector.tensor_add(
                        wt[:, dt, 0:w], pf[:, 0:w], raw[:, dt, 0:w]
                    )

                for dt in range(DT):
                    pending.append(lambda dt=dt: unit(dt))
                pending.append(
                    lambda: nc.gpsimd.dma_start(w1s[g][:, :, 0:w], wt[:, :, 0:w])
                )
                return wt

            def sched_fold_w2(i):
                """Queue fold of W2' row-tile i as 2 half units."""
                raw = w2raw.tile([128, D], BF16, tag="w2raw")
                nc.gpsimd.dma_start(raw[:], w2g[i])
                a2c = b1p.tile([R, 128], BF16, tag="a2c")
                nc.gpsimd.dma_start(a2c[:], a2t[:, i * 128 : (i + 1) * 128])

                def unit(ds):
                    dsl = slice(ds * TS, (ds + 1) * TS)
                    pw = ps.tile([128, TS], F32, tag="pf", bufs=4, name="pwf")
                    nc.tensor.matmul(
                        pw[:], a2c[:], b2_sb[:, dsl], start=True, stop=True
                    )
                    nc.vector.tensor_add(w2sb[:, i, dsl], pw[:], raw[:, dsl])

                for ds in range(2):
                    pending.append(lambda ds=ds: unit(ds))

            def load_w1_group(g):
                w = GDEF[g][1]
                wt = w1c.tile([128, DT, 512], BF16, tag="w1c")
                nc.gpsimd.dma_start(wt[:, :, 0:w], w1s[g][:, :, 0:w])
                return wt

            NGH = NG // 2  # 6 groups per half (gate / up)
            for blk in range(T_CORE // BLK):
                t0 = blk * BLK
                xt = xp.tile([128, DT, BLK], BF16, tag="xt")
                nc.scalar.dma_start(xt[:, 0 : DT // 2, :], xTr[:, 0 : DT // 2, t0 : t0 + BLK])
                nc.gpsimd.dma_start(xt[:, DT // 2 : DT, :], xTr[:, DT // 2 : DT, t0 : t0 + BLK])
                h = hp.tile([128, FFT, BLK], BF16, tag="h")
                gtiles = {}
                # prologue: make group pair 0 available before the f-loop
                if blk == 0:
                    gtiles[0] = sched_fold_w1(0)
                    gtiles[NGH] = sched_fold_w1(NGH)
                    drain(99)
                else:
                    gtiles[0] = load_w1_group(0)
                    gtiles[NGH] = load_w1_group(NGH)
                # ---- phase 1: h = silu(x@W1g') * (x@W1u') ----
                for f in range(FFT):
                    gi, off = (f // 4, (f % 4) * 128) if f < 20 else (5, (f - 20) * 128)
                    if f % 4 == 0 and f // 4 + 1 < NGH:
                        # stage next group pair one pair ahead of use
                        k = f // 4 + 1
                        if blk == 0:
                            gtiles[k] = sched_fold_w1(k)
                            gtiles[NGH + k] = sched_fold_w1(NGH + k)
                        else:
                            gtiles[k] = load_w1_group(k)
                            gtiles[NGH + k] = load_w1_group(NGH + k)
                    if blk == 0:
                        sched_fold_w2(f)
                    gt, ut = gtiles[gi], gtiles[NGH + gi]
                    pg0 = ps.tile([128, TS], F32, tag="pg", bufs=2, name="pg0")
                    pg1 = ps.tile([128, TS], F32, tag="pg", bufs=2, name="pg1")
                    for dt in range(DT):
                        lw = gt[:, dt, off : off + 128]
                        nc.tensor.matmul(pg0[:], lw, xt[:, dt, 0:TS],
                                         start=(dt == 0), stop=(dt == DT - 1))
                        nc.tensor.matmul(pg1[:], lw, xt[:, dt, TS:BLK],
                                         start=(dt == 0), stop=(dt == DT - 1))
                    tmp0 = tmpp.tile([128, TS], BF16, tag="tmp")
                    nc.scalar.activation(tmp0[:], pg0[:], SILU)
                    tmp1 = tmpp.tile([128, TS], BF16, tag="tmp")
                    nc.scalar.activation(tmp1[:], pg1[:], SILU)
                    drain(4)
                    pu0 = ps.tile([128, TS], F32, tag="pu", bufs=2, name="pu0")
                    pu1 = ps.tile([128, TS], F32, tag="pu", bufs=2, name="pu1")
                    for dt in range(DT):
                        lw = ut[:, dt, off : off + 128]
                        nc.tensor.matmul(pu0[:], lw, xt[:, dt, 0:TS],
                                         start=(dt == 0), stop=(dt == DT - 1))
                        nc.tensor.matmul(pu1[:], lw, xt[:, dt, TS:BLK],
                                         start=(dt == 0), stop=(dt == DT - 1))
                    nc.vector.tensor_mul(h[:, f, 0:TS], tmp0[:], pu0[:])
                    nc.vector.tensor_mul(h[:, f, TS:BLK], tmp1[:], pu1[:])
                    drain(4)
                drain(99)
                # ---- phase 2: out = h.T @ W2' ----
                for tt in range(BLK // 128):
                    ttl = slice(tt * 128, (tt + 1) * 128)
                    po0 = ps.tile([128, TS], F32, tag="pf", bufs=4, name="po0")
                    po1 = ps.tile([128, TS], F32, tag="pf", bufs=4, name="po1")
                    for i in range(FFT):
                        lw = h[:, i, ttl]
                        nc.tensor.matmul(po0[:], lw, w2sb[:, i, 0:TS],
                                         start=(i == 0), stop=(i == FFT - 1))
                        nc.tensor.matmul(po1[:], lw, w2sb[:, i, TS:D],
                                         start=(i == 0), stop=(i == FFT - 1))
                    ev = evp.tile([128, D], F32, tag="ev")
                    nc.vector.tensor_copy(ev[:, 0:TS], po0[:])
                    nc.scalar.activation(ev[:, TS:D], po1[:], COPY)
                    nc.scalar.dma_start(out[t0 + tt * 128 : t0 + (tt + 1) * 128, :], ev[:])
    nc.compile()
    return nc


def _get_prog():
    if "nc" not in _prog_cache:
        _prog_cache["nc"] = _build()
    return _prog_cache["nc"]


def run_sharded(inputs, trace=False, tmpdir=None):
    nc = _get_prog()
    x = inputs["x"]
    bf = lambda a: np.ascontiguousarray(a, dtype=BF)
    # group W1 per fold-group [12, 128 partitions, 8 d-tiles, <=512 f-cols]
    # (256-col tail groups zero-padded), W2 as [22 ff-tiles, 128, 1024] so
    # kernel weight DMAs are contiguous per partition
    w1_np = np.asarray(inputs["W_gate_up"])
    w1grp = np.zeros((NG, 128, DT, 512), dtype=BF)
    for g, (s, w) in enumerate(GDEF):
        w1grp[g, :, :, :w] = w1_np[:, s : s + w].reshape(DT, 128, w).transpose(1, 0, 2)
    w2grp = np.asarray(inputs["W_down"]).reshape(FFT, 128, D)
    weights = {
        "W1G": bf(w1grp),
        "B_gate_up": bf(inputs["B_gate_up"]),
        "A1T": bf(np.asarray(inputs["A_gate_up"]).T),
        "W2G": bf(w2grp),
        "A2T": bf(np.asarray(inputs["A_down"]).T),
        "B_down": bf(inputs["B_down"]),
    }
    in_maps = []
    for c in range(N_CORES):
        xs = bf(np.asarray(x[c * T_CORE : (c + 1) * T_CORE]).T)
        in_maps.append({"xT": xs, **weights})
    res = run_bass_kernel_spmd(
        nc, in_maps, list(range(N_CORES)), trace=trace, tmpdir=tmpdir
    )
    outs = [res.results[c]["out"] for c in range(N_CORES)]
    full = np.concatenate(outs, axis=0)
    return full, res


def kernel(**inputs):
    full, _ = run_sharded(inputs, trace=False)
    return full


# revision 34
# speedup vs baseline: 1.0264x; 1.0264x over previous
"""LoRA MLP (gate_up + SiLU*up + down, each with rank-16 LoRA) on 8 TRN2 cores.

Strategy: data-parallel over tokens (16384 = 8 x 2048); weights replicated to
every core, no collectives. All tensors bf16 (PE full rate + FWL fast weight
load + half DMA/SBUF), fp32 PSUM accumulation, fp32 output.

The rank-16 LoRA is folded into the base weights on device:
    W1' = W_gate_up + A_gate_up @ B_gate_up   (PE matmul K=16 + DVE add)
    W2' = W_down    + A_down    @ B_down
so the steady-state loop is a pure dense MLP: clean 8-deep / 22-deep PSUM
accumulation chains with no 16-row LoRA matmuls serializing the PE.

Because the PE instruction queue is strict FIFO, fold work is emitted one
group-pair AHEAD of its consumer and chopped into small units (2 matmuls +
1 DVE add) that are interleaved between the main matmul chunks — by the time
the PE reaches a fold matmul its PSUM slot is long free, so the PE never
stalls on the fold's DVE evacuation chain (which caused HAM re-throttling).

Per core: 2 blocks of 1024 tokens. W2' (44KB/partition) stays SBUF-resident;
W1' groups are folded during block 0 and round-tripped through a DRAM scratch
for block 1 (write and read share one FIFO DMA queue, which orders them).
Activations stay in [feature, token] layout; each LDWEIGHTS feeds 2 matmuls.
DMA spread: weights on sync, x on vector, consts+W2raw on gpsimd, out on
scalar. PSUM: gate 2 + up 2 + shared fold/down-proj 4 banks.
"""

from collections import deque

import numpy as np
import ml_dtypes

import concourse.mybir as mybir
import concourse.tile as tile
from concourse import bacc
from concourse.bass_utils import run_bass_kernel_spmd

TOKENS, D, FF, R = 16384, 1024, 2816, 16
N_CORES = 8
T_CORE = TOKENS // N_CORES  # 2048
BLK = 1024                  # tokens per block (2 blocks/core)
TS = 512                    # psum free-dim tile (1 bank fp32)
DT = D // 128               # 8 d-model tiles
FFT = FF // 128             # 22 ff tiles
# W1' fold groups: per half (gate/up) five 512-col groups + one 256-col tail;
# groups never straddle the gate/up boundary so their lifetimes stay local.
NG = 12
GDEF = [(512 * g, 512) for g in range(5)] + [(2560, 256)]
GDEF += [(2816 + s, w) for s, w in GDEF[:6]]
F32 = mybir.dt.float32
BF16 = mybir.dt.bfloat16
SILU = mybir.ActivationFunctionType.Silu
COPY = mybir.ActivationFunctionType.Copy
BF = ml_dtypes.bfloat16

_prog_cache = {}


def _build():
    nc = bacc.Bacc("TRN2", target_bir_lowering=False, debug=False)
    xT = nc.dram_tensor("xT", [D, T_CORE], BF16, kind="ExternalInput").ap()
    # W1G/W2G are host-pre-grouped so every weight DMA is contiguous per partition
    w1g = nc.dram_tensor("W1G", [NG, 128, DT, 512], BF16, kind="ExternalInput").ap()
    a1t = nc.dram_tensor("A1T", [R, D], BF16, kind="ExternalInput").ap()
    b1 = nc.dram_tensor("B_gate_up", [R, 2 * FF], BF16, kind="ExternalInput").ap()
    w2g = nc.dram_tensor("W2G", [FFT, 128, D], BF16, kind="ExternalInput").ap()
    a2t = nc.dram_tensor("A2T", [R, FF], BF16, kind="ExternalInput").ap()
    b2 = nc.dram_tensor("B_down", [R, D], BF16, kind="ExternalInput").ap()
    out = nc.dram_tensor("out", [T_CORE, D], F32, kind="ExternalOutput").ap()
    # W1' spill space so block 1 re-reads the folded weights instead of refolding
    w1s = nc.dram_tensor("w1s", [NG, 128, DT, 512], BF16, kind="Internal").ap()

    xTr = xT.rearrange("(dt p) t -> p dt t", p=128)   # [128, 8, 2048]

    with tile.TileContext(nc) as tc:
        with (
            tc.tile_pool(name="constp", bufs=1) as constp,
            tc.tile_pool(name="w1c", bufs=4) as w1c,      # W1' group tiles, 8KB each
            tc.tile_pool(name="w2p", bufs=1) as w2p,      # W2' resident
            tc.tile_pool(name="w1raw", bufs=2) as w1raw,
            tc.tile_pool(name="w2raw", bufs=2) as w2raw,
            tc.tile_pool(name="b1p", bufs=3) as b1p,
            tc.tile_pool(name="xp", bufs=2) as xp,
            tc.tile_pool(name="hp", bufs=1) as hp,
            tc.tile_pool(name="tmpp", bufs=2) as tmpp,
            tc.tile_pool(name="evp", bufs=2) as evp,
            tc.tile_pool(name="ps", bufs=1, space="PSUM") as ps,
        ):
            a1t_sb = constp.tile([R, D], BF16)
            nc.gpsimd.dma_start(a1t_sb[:], a1t[:])
            b2_sb = constp.tile([R, D], BF16)
            nc.gpsimd.dma_start(b2_sb[:], b2[:])
            w2sb = w2p.tile([128, FFT, D], BF16)

            pending = deque()  # fold micro-units, drained between main MM chunks

            def drain(n):
                for _ in range(min(n, len(pending))):
                    pending.popleft()()

            def sched_fold_w1(g):
                """Queue fold of W1' group g (8 units of one dt each + spill)."""
                c0, w = GDEF[g]
                raw = w1raw.tile([128, DT, 512], BF16, tag="w1raw")
                nc.sync.dma_start(raw[:, :, 0:w], w1g[g][:, :, 0:w])
                b1c = b1p.tile([R, 512], BF16, tag="b1c")
                nc.sync.dma_start(b1c[:, 0:w], b1[:, c0 : c0 + w])
                wt = w1c.tile([128, DT, 512], BF16, tag="w1c")

                def unit(dt):
                    pf = ps.tile([128, TS], F32, tag="pf", bufs=4, name="pf")
                    nc.tensor.matmul(
                        pf[:, 0:w],
                        a1t_sb[:, dt * 128 : (dt + 1) * 128],
                        b1c[:, 0:w],
                        start=True, stop=True,
                    )
                    nc.vector.tensor_add(
                        wt[:, dt, 0:w], pf[:, 0:w], raw[:, dt, 0:w]
                    )

                for dt in range(DT):
                    pending.append(lambda dt=dt: unit(dt))
                pending.append(
                    lambda: nc.gpsimd.dma_start(w1s[g][:, :, 0:w], wt[:, :, 0:w])
                )
                return wt

            def sched_fold_w2(i):
                """Queue fold of W2' row-tile i as 2 half units."""
                raw = w2raw.tile([128, D], BF16, tag="w2raw")
                nc.gpsimd.dma_start(raw[:], w2g[i])
                a2c = b1p.tile([R, 128], BF16, tag="a2c")
                nc.gpsimd.dma_start(a2c[:], a2t[:, i * 128 : (i + 1) * 128])

                def unit(ds):
                    dsl = slice(ds * TS, (ds + 1) * TS)
                    pw = ps.tile([128, TS], F32, tag="pf", bufs=4, name="pwf")
                    nc.tensor.matmul(
                        pw[:], a2c[:], b2_sb[:, dsl], start=True, stop=True
                    )
                    nc.vector.tensor_add(w2sb[:, i, dsl], pw[:], raw[:, dsl])

                for ds in range(2):
                    pending.append(lambda ds=ds: unit(ds))

            def load_w1_group(g):
                w = GDEF[g][1]
                wt = w1c.tile([128, DT, 512], BF16, tag="w1c")
                nc.gpsimd.dma_start(wt[:, :, 0:w], w1s[g][:, :, 0:w])
                return wt

            NGH = NG // 2  # 6 groups per half (gate / up)
            for blk in range(T_CORE // BLK):
                t0 = blk * BLK
                xt = xp.tile([128, DT, BLK], BF16, tag="xt")
                nc.scalar.dma_start(xt[:, 0 : DT // 2, :], xTr[:, 0 : DT // 2, t0 : t0 + BLK])
                nc.gpsimd.dma_start(xt[:, DT // 2 : DT, :], xTr[:, DT // 2 : DT, t0 : t0 + BLK])
                h = hp.tile([128, FFT, BLK], BF16, tag="h")
                gtiles = {}
                # prologue: make group pair 0 available before the f-loop
                if blk == 0:
                    gtiles[0] = sched_fold_w1(0)
                    gtiles[NGH] = sched_fold_w1(NGH)
                    drain(99)
                else:
                    gtiles[0] = load_w1_group(0)
                    gtiles[NGH] = load_w1_group(NGH)
                # ---- phase 1: h = silu(x@W1g') * (x@W1u') ----
                for f in range(FFT):
                    gi, off = (f // 4, (f % 4) * 128) if f < 20 else (5, (f - 20) * 128)
                    # stage next groups one group-pair ahead of use, staggered
                    # (gate at f%4==0, up at f%4==2) to halve fold-unit bursts
                    if f % 4 == 0 and f // 4 + 1 < NGH:
                        k = f // 4 + 1
                        gtiles[k] = sched_fold_w1(k) if blk == 0 else load_w1_group(k)
                    if f % 4 == 2 and f // 4 + 1 < NGH:
                        k = NGH + f // 4 + 1
                        gtiles[k] = sched_fold_w1(k) if blk == 0 else load_w1_group(k)
                    if blk == 0:
                        sched_fold_w2(f)
                    gt, ut = gtiles[gi], gtiles[NGH + gi]
                    pg0 = ps.tile([128, TS], F32, tag="pg", bufs=2, name="pg0")
                    pg1 = ps.tile([128, TS], F32, tag="pg", bufs=2, name="pg1")
                    for dt in range(DT):
                        lw = gt[:, dt, off : off + 128]
                        nc.tensor.matmul(pg0[:], lw, xt[:, dt, 0:TS],
                                         start=(dt == 0), stop=(dt == DT - 1))
                        nc.tensor.matmul(pg1[:], lw, xt[:, dt, TS:BLK],
                                         start=(dt == 0), stop=(dt == DT - 1))
                    tmp0 = tmpp.tile([128, TS], BF16, tag="tmp")
                    nc.scalar.activation(tmp0[:], pg0[:], SILU)
                    tmp1 = tmpp.tile([128, TS], BF16, tag="tmp")
                    nc.scalar.activation(tmp1[:], pg1[:], SILU)
                    drain(4)
                    pu0 = ps.tile([128, TS], F32, tag="pu", bufs=2, name="pu0")
                    pu1 = ps.tile([128, TS], F32, tag="pu", bufs=2, name="pu1")
                    for dt in range(DT):
                        lw = ut[:, dt, off : off + 128]
                        nc.tensor.matmul(pu0[:], lw, xt[:, dt, 0:TS],
                                         start=(dt == 0), stop=(dt == DT - 1))
                        nc.tensor.matmul(pu1[:], lw, xt[:, dt, TS:BLK],
                                         start=(dt == 0), stop=(dt == DT - 1))
                    nc.vector.tensor_mul(h[:, f, 0:TS], tmp0[:], pu0[:])
                    nc.vector.tensor_mul(h[:, f, TS:BLK], tmp1[:], pu1[:])
                    drain(4)
                drain(99)
                # ---- phase 2: out = h.T @ W2' ----
                for tt in range(BLK // 128):
                    ttl = slice(tt * 128, (tt + 1) * 128)
                    po0 = ps.tile([128, TS], F32, tag="pf", bufs=4, name="po0")
                    po1 = ps.tile([128, TS], F32, tag="pf", bufs=4, name="po1")
                    for i in range(FFT):
                        lw = h[:, i, ttl]
                        nc.tensor.matmul(po0[:], lw, w2sb[:, i, 0:TS],
                                         start=(i == 0), stop=(i == FFT - 1))
                        nc.tensor.matmul(po1[:], lw, w2sb[:, i, TS:D],
                                         start=(i == 0), stop=(i == FFT - 1))
                    ev = evp.tile([128, D], F32, tag="ev")
                    nc.vector.tensor_copy(ev[:, 0:TS], po0[:])
                    nc.scalar.activation(ev[:, TS:D], po1[:], COPY)
                    nc.scalar.dma_start(out[t0 + tt * 128 : t0 + (tt + 1) * 128, :], ev[:])
    nc.compile()
    return nc


def _get_prog():
    if "nc" not in _prog_cache:
        _prog_cache["nc"] = _build()
    return _prog_cache["nc"]


def run_sharded(inputs, trace=False, tmpdir=None):
    nc = _get_prog()
    x = inputs["x"]
    bf = lambda a: np.ascontiguousarray(a, dtype=BF)
    # group W1 per fold-group [12, 128 partitions, 8 d-tiles, <=512 f-cols]
    # (256-col tail groups zero-padded), W2 as [22 ff-tiles, 128, 1024] so
    # kernel weight DMAs are contiguous per partition
    w1_np = np.asarray(inputs["W_gate_up"])
    w1grp = np.zeros((NG, 128, DT, 512), dtype=BF)
    for g, (s, w) in enumerate(GDEF):
        w1grp[g, :, :, :w] = w1_np[:, s : s + w].reshape(DT, 128, w).transpose(1, 0, 2)
    w2grp = np.asarray(inputs["W_down"]).reshape(FFT, 128, D)
    weights = {
        "W1G": bf(w1grp),
        "B_gate_up": bf(inputs["B_gate_up"]),
        "A1T": bf(np.asarray(inputs["A_gate_up"]).T),
        "W2G": bf(w2grp),
        "A2T": bf(np.asarray(inputs["A_down"]).T),
        "B_down": bf(inputs["B_down"]),
    }
    in_maps = []
    for c in range(N_CORES):
        xs = bf(np.asarray(x[c * T_CORE : (c + 1) * T_CORE]).T)
        in_maps.append({"xT": xs, **weights})
    res = run_bass_kernel_spmd(
        nc, in_maps, list(range(N_CORES)), trace=trace, tmpdir=tmpdir
    )
    outs = [res.results[c]["out"] for c in range(N_CORES)]
    full = np.concatenate(outs, axis=0)
    return full, res


def kernel(**inputs):
    full, _ = run_sharded(inputs, trace=False)
    return full
